# revision 15
# baseline (speedup 1.0000x reference)
"""Multi-head dense attention (no softmax) on 8 Trainium2 NeuronCores.

Math (per batch b, head h with head_dim d=64):
    q   = x @ W^T                      # [S, H] projection
    out_h = (q_h x_h^T) x_h            # naive: O(S^2 d) with an SxS temp
          = q_h (x_h^T x_h)            # reassociated: Gram matrix G_h [d, d]
The reassociation is exact (same sum, different order) and collapses the
FLOPs ~5x while removing the SxS intermediate entirely.

Sharding: core c handles batch b = c//2 and head-group hg = c%2 (8 heads,
512 output columns). Cores are fully independent (no collectives).

Device layout per core (all inputs fp16; W is pre-scaled by 1024 on the
host so its sigma~9e-5 entries clear fp16's subnormal cutoff, and the
output copy multiplies by 1/1024 to undo it):
    xT  [1024, 2048]  x[b] transposed (host-prepped) - projection operands
    xn  [2048, 512]   x[b] natural, this head-group's columns - Gram operands
    wT  [1024, 512]   1024 * W rows of this head-group, transposed (k-major)
    outT [512, 2048]  output transposed (fp32); host transposes back

Stages (fp16 matmuls: full PE rate, FWL fast weight loads, fp32 PSUM):
    proj: qT' = (1024 W) x^T accumulated over k-tiles, sc-inner loop order
          so the first m-tile's matmuls pace with xT's DMA arrival.
    G:    per head-pair p: psum += xn_p^T @ xn_p over 16 s-tiles, then copy
          the two 64x64 diagonal blocks into a zeroed block-diag fp16 tile.
    out:  outT'[pair, chunk] = Gbd^T @ qT'_pair (G symmetric; one N=512
          matmul per pair/chunk via the block-diagonal trick), then the
          PSUM->SBUF copy scales by 1/1024.
"""

import numpy as np

B, S, H = 4, 2048, 1024
N_HEADS = 16
HD = H // N_HEADS  # 64
N_CORES = 8
MG = H // 2        # 512 output columns per core
P = 128
KT = H // P        # 8 k-tiles
ST = S // P        # 16 s-tiles
MT = MG // P       # 4 m-tiles == head pairs
SC = S // 512      # 4 s-chunks
W_SCALE = 1024.0

_NC_CACHE = {}


def _build_nc():
    import concourse.mybir as mybir
    from concourse import bacc
    from concourse.tile import TileContext

    f32 = mybir.dt.float32
    f16 = mybir.dt.float16

    nc = bacc.Bacc()
    xT_d = nc.declare_dram_parameter("xT", [H, S], f16, isOutput=False)
    xn_d = nc.declare_dram_parameter("xn", [S, MG], f16, isOutput=False)
    wT_d = nc.declare_dram_parameter("wT", [H, MG], f16, isOutput=False)
    outT_d = nc.declare_dram_parameter("outT", [MG, S], f16, isOutput=True)

    xT_t = xT_d.rearrange("(kt p) s -> p kt s", p=P)   # [128, 8, 2048]
    xn_t = xn_d.rearrange("(st p) m -> p st m", p=P)   # [128, 16, 512]
    wT_t = wT_d.rearrange("(kt p) m -> p kt m", p=P)   # [128, 8, 512]

    with TileContext(nc) as tc:
        with (
            tc.tile_pool(name="big", bufs=1) as big,
            tc.tile_pool(name="gp", bufs=1) as gpool,
            tc.tile_pool(name="stage", bufs=4) as stage,
            tc.tile_pool(name="ps_q", bufs=1, space="PSUM") as ps_q,
            tc.tile_pool(name="ps_g", bufs=2, space="PSUM") as ps_g,
            tc.tile_pool(name="ps_o", bufs=2, space="PSUM") as ps_o,
        ):
            xT_sb = big.tile([P, KT, S], f16, tag="xT")
            xn_sb = big.tile([P, ST, MG], f16, tag="xn")
            wT_sb = big.tile([P, KT, MG], f16, tag="wT")
            qT_sb = big.tile([P, MT, S], f16, tag="qT")

            # All inputs stream on the Activation engine's DMA ring in
            # exact consumption order (a single ring preserves ordering and
            # still saturates ~300 GB/s): xn first (Gram runs while xT
            # arrives), then wT, then xT k-tiles. Output stores use the
            # Sync ring so they never queue behind input descriptors.
            for st in range(ST):
                nc.scalar.dma_start(out=xn_sb[:, st], in_=xn_t[:, st])
            nc.scalar.dma_start(out=wT_sb, in_=wT_t)
            for kt in range(KT):
                nc.scalar.dma_start(out=xT_sb[:, kt], in_=xT_t[:, kt])

            # ---- Gram stage first: needs only xn, runs while xT streams in.
            gbd = []
            for p_i in range(MT):
                psg = ps_g.tile([P, P], f32, tag="psg", name=f"psg{p_i}")
                xp = xn_sb[:, :, p_i * P:(p_i + 1) * P]
                for i in range(ST):
                    nc.tensor.matmul(
                        psg,
                        lhsT=xp[:, i],
                        rhs=xp[:, i],
                        start=(i == 0),
                        stop=(i == ST - 1),
                    )
                g = gpool.tile([P, P], f16, tag=f"g{p_i}", name=f"g{p_i}")
                nc.vector.memset(g, 0.0)
                nc.vector.tensor_scalar_mul(
                    out=g[0:HD, 0:HD], in0=psg[0:HD, 0:HD], scalar1=1.0 / W_SCALE
                )
                nc.vector.tensor_scalar_mul(
                    out=g[HD:P, HD:P], in0=psg[HD:P, HD:P], scalar1=1.0 / W_SCALE
                )
                gbd.append(g)

            # ---- Projection per m-tile, with pair p's output stage emitted
            # one m-tile later so its qT copies have already drained.
            def emit_out(p_i):
                for sc in range(SC):
                    pso = ps_o.tile([P, 512], f32, tag="pso", name=f"pso{p_i}_{sc}")
                    nc.tensor.matmul(
                        pso,
                        lhsT=gbd[p_i],
                        rhs=qT_sb[:, p_i, sc * 512:(sc + 1) * 512],
                        start=True,
                        stop=True,
                    )
                    ot = stage.tile([P, 512], f16, tag="ot", name=f"ot{p_i}_{sc}")
                    nc.vector.tensor_copy(out=ot, in_=pso)
                    nc.sync.dma_start(
                        out=outT_d[p_i * P:(p_i + 1) * P, sc * 512:(sc + 1) * 512],
                        in_=ot,
                    )

            for mt in range(MT):
                psqs = [
                    ps_q.tile([P, 512], f32, tag=f"psq{sc}", name=f"psq{mt}_{sc}")
                    for sc in range(SC)
                ]
                for kt in range(KT):
                    for sc in range(SC):
                        nc.tensor.matmul(
                            psqs[sc],
                            lhsT=wT_sb[:, kt, mt * P:(mt + 1) * P],
                            rhs=xT_sb[:, kt, sc * 512:(sc + 1) * 512],
                            start=(kt == 0),
                            stop=(kt == KT - 1),
                        )
                for sc in range(SC):
                    nc.vector.tensor_copy(
                        out=qT_sb[:, mt, sc * 512:(sc + 1) * 512], in_=psqs[sc]
                    )
                if mt >= 1:
                    emit_out(mt - 1)
            emit_out(MT - 1)
    nc.compile()
    return nc


def _get_nc():
    if "nc" not in _NC_CACHE:
        _NC_CACHE["nc"] = _build_nc()
    return _NC_CACHE["nc"]


def make_in_maps(hidden_states, queries_weight):
    hs = np.ascontiguousarray(np.asarray(hidden_states, dtype=np.float32))
    w = np.ascontiguousarray(np.asarray(queries_weight, dtype=np.float32))
    in_maps = []
    for c in range(N_CORES):
        b, hg = divmod(c, 2)
        xb = hs[b]
        in_maps.append({
            "xT": np.ascontiguousarray(xb.T).astype(np.float16),
            "xn": np.ascontiguousarray(xb[:, hg * MG:(hg + 1) * MG]).astype(
                np.float16
            ),
            "wT": np.ascontiguousarray(
                w[hg * MG:(hg + 1) * MG, :].T * W_SCALE
            ).astype(np.float16),
        })
    return in_maps


def assemble_output(results):
    out = np.empty((B, S, H), dtype=np.float32)
    for c in range(N_CORES):
        b, hg = divmod(c, 2)
        out[b, :, hg * MG:(hg + 1) * MG] = results[c]["outT"].T.astype(np.float32)
    return out


def kernel(hidden_states, queries_weight):
    from concourse.bass_utils import run_bass_kernel_spmd

    in_maps = make_in_maps(hidden_states, queries_weight)
    res = run_bass_kernel_spmd(
        _get_nc(), in_maps, core_ids=list(range(N_CORES))
    ).results
    return assemble_output(res)


if __name__ == "__main__":
    x = np.random.randn(B, S, H).astype(np.float32)
    w = np.random.randn(H, H).astype(np.float32) * 1e-4
    out = kernel(x, w)
    print(out.shape, out.dtype)
